# revision 9
# baseline (speedup 1.0000x reference)
"""BitSwiGLU Trainium2 kernel (8 NeuronCores, data-parallel over tokens).

Math (per bit_linear, forward values):
    gamma_x = clip(max|x_row|, 1e-5);  k = rne(x * 127/gamma_x)  in [-127,127]
    gamma_w = clip(mean|w|, 1e-5);    t = sign(w) * (|w| > 0.5*gamma_w)  in {-1,0,1}
    y = (k @ t.T) * (gamma_x*gamma_w/127) + b

k and t are small integers, exactly representable in bf16; the TensorEngine
accumulates bf16 products in fp32 PSUM, so k @ t.T is EXACT integer math at
bf16 speed. All scales are applied per-token (per-partition) at PSUM eviction.

Ternarization runs as t2 = sign(w - thr) + sign(w + thr) in {-2,0,2}
(two ScalarE Sign ops + one bf16 VectorE add; fp32 subtract-sign is exact,
so the comparison against thr = 0.5*gamma is bit-exact). The factor 2 is
folded into the eviction scales (exact power of two).

Sharding: data-parallel — 8192 tokens split 1024/core; weights replicated.
Each core ternarizes weights locally, writes them to DRAM as bf16 in
natural layout, and the matmul phases transpose-load [K,512] tiles through
the DMA XBAR.

silu(y) is computed as y * sigmoid(y) (Sigmoid on ScalarE).
Biases are zero in this problem; gate/val biases are asserted zero host-side
and out_b is added on host.
"""

import numpy as np

import concourse.bass as bass
import concourse.mybir as mybir
import concourse.tile as tile
from concourse import bacc
from concourse import bass_isa
from concourse.bass_utils import run_bass_kernel_spmd

F32 = mybir.dt.float32
BF16 = mybir.dt.bfloat16
AF = mybir.ActivationFunctionType
OP = mybir.AluOpType
AX = mybir.AxisListType

MAGIC = 12582912.0  # 1.5 * 2**23 : (v + MAGIC) - MAGIC == rne(v) for |v| < 2**22

N_CORES = 8


def _build(T, D, H, n_cores=N_CORES):
    """Build + compile the per-core Bass program. All cores run the same
    program on their own token shard (weights replicated)."""
    nc = bacc.Bacc("TRN2", target_bir_lowering=False, debug=False,
                   num_devices=n_cores)
    x_d = nc.dram_tensor("x", [T, D], F32, kind="ExternalInput")
    gw_d = nc.dram_tensor("gate_w", [H, D], F32, kind="ExternalInput")
    vw_d = nc.dram_tensor("val_w", [H, D], F32, kind="ExternalInput")
    ow_d = nc.dram_tensor("out_w", [D, H], F32, kind="ExternalInput")
    out_d = nc.dram_tensor("out", [T, D], F32, kind="ExternalOutput")

    with tile.TileContext(nc) as tc:
        _body(tc, x_d, gw_d, vw_d, ow_d, out_d, T=T, D=D, H=H)
    nc.compile()
    return nc


def _body(tc, x_d, gw_d, vw_d, ow_d, out_d, *, T, D, H):
    nc = tc.nc
    KD = D // 128      # contraction chunks, mm1
    KH = H // 128      # contraction chunks, mm2
    NH = H // 512      # hidden 512-chunks (mm1 output tiles)
    ND = D // 512      # d_out 512-chunks (mm2 output tiles)
    MT = T // 128      # token chunks
    RG = H // 128      # gate/val weight row-chunks
    RO = D // 128      # out_w row-chunks
    CW = min(2048, D)  # gate/val weight processing width
    NW = D // CW
    CO = min(2048, H)  # out_w weight processing width
    NO = H // CO
    CQ = min(2048, H)  # h-quant processing chunk
    NQ = H // CQ
    MHALF = max(1, MT // 2)

    Xv = x_d.ap().rearrange("(m p) d -> m p d", p=128)
    Ov = out_d.ap().rearrange("(m p) d -> m p d", p=128)

    with (
        tc.tile_pool(name="persist", bufs=1) as pp,
        tc.tile_pool(name="psp", bufs=8, space="PSUM") as psp,
        tc.tile_pool(name="drp", bufs=1, space="DRAM") as drp,
    ):
        # DRAM scratch: pre-transposed ternary out_w chunks + h
        oq_d = drp.tile([H // 128, D // 128, 128, 128], BF16, tag="oq")
        h_d = drp.tile([MT, 128, H], F32, tag="h")

        s1, s12, gx_l, hmax = [], [], [], []
        for m in range(MT):
            for nm, lst in (("s1", s1), ("s12", s12), ("gx", gx_l),
                            ("hmax", hmax)):
                t = pp.tile([128, 1], F32, tag=f"{nm}{m}", name=f"{nm}{m}")
                lst.append(t)
        hp = [pp.tile([128, NH], F32, tag=f"hp{m}", name=f"hp{m}")
              for m in range(MT)]

        with tc.tile_pool(name="kxp", bufs=1) as kxp:
            # ---------------- x quantization + transpose ----------------
            # kxT[p=d, k, t] = k_x[t, k*128+p]
            kxT = kxp.tile([128, KD, T], BF16, tag="kxT")
            with tc.tile_pool(name="xst", bufs=3) as xst:
                for m in range(MT):
                    xt = xst.tile([128, D], F32, tag="x_in")
                    nc.sync.dma_start(out=xt[:, :], in_=Xv[m])
                    gx = gx_l[m]
                    nc.vector.tensor_reduce(out=gx[:, :], in_=xt[:, :],
                                            axis=AX.X, op=OP.max,
                                            apply_absolute_value=True)
                    nc.vector.tensor_scalar_max(out=gx[:, :], in0=gx[:, :],
                                                scalar1=1e-5)
                    rcp = xst.tile([128, 1], F32, tag="rcpx")
                    nc.vector.reciprocal(out=rcp[:, :], in_=gx[:, :])
                    sx = xst.tile([128, 1], F32, tag="sx")
                    nc.vector.tensor_scalar_mul(out=sx[:, :], in0=rcp[:, :],
                                                scalar1=127.0)
                    # k_x = rne(x * sx) -> bf16 (exact small ints)
                    xs = xst.tile([128, D], F32, tag="x_sc")
                    nc.scalar.activation(out=xs[:, :], in_=xt[:, :],
                                         func=AF.Copy, scale=sx[:, :])
                    kx = xst.tile([128, D], BF16, tag="kx")
                    nc.vector.tensor_scalar(out=kx[:, :], in0=xs[:, :],
                                            scalar1=MAGIC, scalar2=MAGIC,
                                            op0=OP.add, op1=OP.subtract)
                    nc.sync.dma_start(out=kxT[:, :, m * 128:(m + 1) * 128],
                                      in_=kx[:, :], transpose=True)

            # ---------------- weight gammas ----------------
            with tc.tile_pool(name="gp", bufs=3) as gp:
                def gamma_of(w_ap, R, C, NC_, label, engine):
                    CWc = C // NC_
                    Wv = w_ap.rearrange("(r p) c -> r p c", p=128)
                    parts = pp.tile([128, R * NC_], F32, tag=f"parts_{label}",
                                    name=f"parts_{label}")
                    for r in range(R):
                        for j in range(NC_):
                            wt = gp.tile([128, CWc], F32, tag="g_in")
                            nc.sync.dma_start(
                                out=wt[:, :],
                                in_=Wv[r][:, j * CWc:(j + 1) * CWc])
                            col = parts[:, r * NC_ + j:r * NC_ + j + 1]
                            if engine == "dve":
                                nc.vector.tensor_reduce(
                                    out=col, in_=wt[:, :], axis=AX.X,
                                    op=OP.add, apply_absolute_value=True)
                            else:
                                scr = gp.tile([128, CWc], F32, tag="g_scr")
                                nc.scalar.activation(
                                    out=scr[:, :], in_=wt[:, :], func=AF.Abs,
                                    accum_out=col)
                    tot = pp.tile([128, 1], F32, tag=f"gsum_{label}",
                                  name=f"gsum_{label}")
                    nc.vector.tensor_reduce(out=tot[:, :], in_=parts[:, :],
                                            axis=AX.X, op=OP.add)
                    nc.gpsimd.partition_all_reduce(tot[:, :], tot[:, :], 128,
                                                   bass_isa.ReduceOp.add)
                    g = pp.tile([128, 1], F32, tag=f"gamma_{label}",
                                name=f"gamma_{label}")
                    nc.vector.tensor_scalar(out=g[:, :], in0=tot[:, :],
                                            scalar1=1.0 / (R * 128 * C),
                                            scalar2=1e-5, op0=OP.mult,
                                            op1=OP.max)
                    thr = pp.tile([128, 1], F32, tag=f"thr_{label}",
                                  name=f"thr_{label}")
                    nc.vector.tensor_scalar_mul(out=thr[:, :], in0=g[:, :],
                                                scalar1=0.5)
                    nthr = pp.tile([128, 1], F32, tag=f"nthr_{label}",
                                   name=f"nthr_{label}")
                    nc.vector.tensor_scalar_mul(out=nthr[:, :], in0=thr[:, :],
                                                scalar1=-1.0)
                    return g, thr, nthr

                g_gw, thr_g, nthr_g = gamma_of(gw_d.ap(), RG, D, NW, "g",
                                               "dve")
                g_vw, thr_v, nthr_v = gamma_of(vw_d.ap(), RG, D, NW, "v",
                                               "act")

                # per-token eviction scales; /254 folds the ternary 2x
                for m in range(MT):
                    nc.vector.tensor_scalar(out=s1[m][:, :],
                                            in0=gx_l[m][:, :],
                                            scalar1=g_gw[:, :],
                                            scalar2=1.0 / 254.0,
                                            op0=OP.mult, op1=OP.mult)
                    s2 = gp.tile([128, 1], F32, tag="s2tmp")
                    nc.vector.tensor_scalar(out=s2[:, :], in0=gx_l[m][:, :],
                                            scalar1=g_vw[:, :],
                                            scalar2=1.0 / 254.0,
                                            op0=OP.mult, op1=OP.mult)
                    nc.vector.tensor_mul(out=s12[m][:, :], in0=s1[m][:, :],
                                         in1=s2[:, :])

                g_ow, thr_o, nthr_o = gamma_of(ow_d.ap(), RO, H, NO, "o",
                                               "dve")

            # ---------------- fused ternarize + mm1 ----------------
            # ternarize: t2 = sign(w-thr) + sign(w+thr) in {-2,0,2}; gate/val
            # rows for hidden-slice n go straight to SBUF via XBAR transpose.
            RPN = 512 // 128  # weight row-chunks per mm1 hidden slice
            Gv = gw_d.ap().rearrange("(r p) c -> r p c", p=128)
            Vv = vw_d.ap().rearrange("(r p) c -> r p c", p=128)
            with tc.tile_pool(name="m1p", bufs=2) as m1p:
                def tern(Wv, r, j, CWc, thr, nthr, out_ap):
                    sl = slice(j * CWc, (j + 1) * CWc)
                    wt = m1p.tile([128, CWc], F32, tag="q_in", bufs=3)
                    nc.sync.dma_start(out=wt[:, :], in_=Wv[r][:, sl])
                    sp = m1p.tile([128, CWc], BF16, tag="q_sp", bufs=2)
                    nc.scalar.activation(out=sp[:, :], in_=wt[:, :],
                                         func=AF.Sign, bias=nthr[:, :])
                    sn = m1p.tile([128, CWc], BF16, tag="q_sn", bufs=2)
                    nc.scalar.activation(out=sn[:, :], in_=wt[:, :],
                                         func=AF.Sign, bias=thr[:, :])
                    tq = m1p.tile([128, CWc], BF16, tag="q_tq", bufs=2)
                    nc.vector.tensor_add(out=tq[:, :], in0=sp[:, :],
                                         in1=sn[:, :])
                    # out[p, k, f] = tq[f, k*128+p]
                    nc.sync.dma_start(out=out_ap, in_=tq[:, :],
                                      transpose=True)

                Owv = ow_d.ap().rearrange("(r p) c -> r p c", p=128)
                KO = CO // 128
                ow_units = [(r, j) for r in range(RO) for j in range(NO)]

                def ow_quant_unit(r, j):
                    sl = slice(j * CO, (j + 1) * CO)
                    wt = m1p.tile([128, CO], F32, tag="q_in", bufs=3,
                                  name=f"owt{r}_{j}")
                    nc.sync.dma_start(out=wt[:, :], in_=Owv[r][:, sl])
                    sp = m1p.tile([128, CO], BF16, tag="q_sp", bufs=2,
                                  name=f"owsp{r}_{j}")
                    nc.scalar.activation(out=sp[:, :], in_=wt[:, :],
                                         func=AF.Sign, bias=nthr_o[:, :])
                    sn = m1p.tile([128, CO], BF16, tag="q_sn", bufs=2,
                                  name=f"owsn{r}_{j}")
                    nc.scalar.activation(out=sn[:, :], in_=wt[:, :],
                                         func=AF.Sign, bias=thr_o[:, :])
                    tq = m1p.tile([128, CO], BF16, tag="q_tq", bufs=2,
                                  name=f"owtq{r}_{j}")
                    nc.vector.tensor_add(out=tq[:, :], in0=sp[:, :],
                                         in1=sn[:, :])
                    tT = m1p.tile([128, KO, 128], BF16, tag="q_tT", bufs=2,
                                  name=f"owtT{r}_{j}")
                    nc.sync.dma_start(out=tT[:, :, :], in_=tq[:, :],
                                      transpose=True)
                    for kk in range(KO):
                        nc.sync.dma_start(out=oq_d[j * KO + kk, r],
                                          in_=tT[:, kk, :])

                for n in range(NH):
                    wg_n = m1p.tile([128, KD, 512], BF16, tag="wg_n")
                    wv_n = m1p.tile([128, KD, 512], BF16, tag="wv_n")
                    for rr in range(RPN):
                        r = RPN * n + rr
                        for j in range(NW):
                            fs = slice(rr * 128, (rr + 1) * 128)
                            tern(Gv, r, j, CW, thr_g, nthr_g,
                                 wg_n[:, j * (CW // 128):(j + 1) * (CW // 128),
                                      fs] if NW > 1 else wg_n[:, :, fs])
                            tern(Vv, r, j, CW, thr_v, nthr_v,
                                 wv_n[:, j * (CW // 128):(j + 1) * (CW // 128),
                                      fs] if NW > 1 else wv_n[:, :, fs])
                    for half in range(MT // MHALF):
                        ms = range(half * MHALF, (half + 1) * MHALF)
                        pg = {m: psp.tile([128, 512], F32, tag="ps",
                                          name=f"pg{n}_{m}") for m in ms}
                        pv = {m: psp.tile([128, 512], F32, tag="ps",
                                          name=f"pv{n}_{m}") for m in ms}
                        for k in range(KD):
                            for m in ms:
                                lhsT = kxT[:, k, m * 128:(m + 1) * 128]
                                nc.tensor.matmul(pg[m][:, :], lhsT=lhsT,
                                                 rhs=wg_n[:, k, :],
                                                 start=(k == 0),
                                                 stop=(k == KD - 1))
                                nc.tensor.matmul(pv[m][:, :], lhsT=lhsT,
                                                 rhs=wv_n[:, k, :],
                                                 start=(k == 0),
                                                 stop=(k == KD - 1))
                        for m in ms:
                            A = m1p.tile([128, 512], F32, tag="Asb",
                                         bufs=MHALF + 1, name=f"A{n}_{m}")
                            nc.scalar.activation(out=A[:, :], in_=pg[m][:, :],
                                                 func=AF.Sigmoid,
                                                 scale=s1[m][:, :])
                            B = m1p.tile([128, 512], F32, tag="Bsb",
                                         bufs=MHALF + 1, name=f"B{n}_{m}")
                            nc.scalar.activation(out=B[:, :], in_=pg[m][:, :],
                                                 func=AF.Copy,
                                                 scale=s12[m][:, :])
                            tmp = m1p.tile([128, 512], F32, tag="tmp", bufs=3,
                                           name=f"tmp{n}_{m}")
                            nc.vector.tensor_mul(out=tmp[:, :],
                                                 in0=pv[m][:, :],
                                                 in1=B[:, :])
                            hs = m1p.tile([128, 512], F32, tag="hsl", bufs=3,
                                          name=f"hs{n}_{m}")
                            nc.vector.tensor_mul(out=hs[:, :], in0=A[:, :],
                                                 in1=tmp[:, :])
                            nc.vector.tensor_reduce(
                                out=hp[m][:, n:n + 1], in_=hs[:, :],
                                axis=AX.X, op=OP.max,
                                apply_absolute_value=True)
                            nc.sync.dma_start(
                                out=h_d[m, :, n * 512:(n + 1) * 512],
                                in_=hs[:, :])
                    lo = (len(ow_units) * n) // NH
                    hi = (len(ow_units) * (n + 1)) // NH
                    for (r_, j_) in ow_units[lo:hi]:
                        ow_quant_unit(r_, j_)


        # ---------------- h quantization + mm2 (pipelined by token pair) ----
        with tc.tile_pool(name="khp", bufs=1) as khp:
            with (
                tc.tile_pool(name="hqp", bufs=3) as hqp,
                tc.tile_pool(name="m2p", bufs=3) as m2p,
            ):
                def kh_quant(m):
                    nc.vector.tensor_reduce(out=hmax[m][:, :],
                                            in_=hp[m][:, :], axis=AX.X,
                                            op=OP.max)
                    gh = hqp.tile([128, 1], F32, tag="gh", name=f"gh{m}")
                    nc.vector.tensor_scalar_max(out=gh[:, :],
                                                in0=hmax[m][:, :],
                                                scalar1=1e-5)
                    rch = hqp.tile([128, 1], F32, tag="rch", name=f"rch{m}")
                    nc.vector.reciprocal(out=rch[:, :], in_=gh[:, :])
                    sh = hqp.tile([128, 1], F32, tag="sh", name=f"sh{m}")
                    nc.vector.tensor_scalar_mul(out=sh[:, :], in0=rch[:, :],
                                                scalar1=127.0)
                    so = pp.tile([128, 1], F32, tag=f"so{m}", name=f"so{m}")
                    nc.vector.tensor_scalar(out=so[:, :], in0=gh[:, :],
                                            scalar1=g_ow[:, :],
                                            scalar2=1.0 / 254.0,
                                            op0=OP.mult, op1=OP.mult)
                    kT = khp.tile([128, KH, 128], BF16, tag="khT", bufs=4,
                                  name=f"khT{m}")
                    for q in range(NQ):
                        hc = hqp.tile([128, CQ], F32, tag="h_rd",
                                      name=f"hc{m}_{q}")
                        nc.sync.dma_start(out=hc[:, :],
                                          in_=h_d[m, :, q * CQ:(q + 1) * CQ])
                        hsc = hqp.tile([128, CQ], F32, tag="h_sc",
                                       name=f"hsc{m}_{q}")
                        nc.scalar.activation(out=hsc[:, :], in_=hc[:, :],
                                             func=AF.Copy, scale=sh[:, :])
                        kh = hqp.tile([128, CQ], BF16, tag="kh",
                                      name=f"kh{m}_{q}")
                        nc.vector.tensor_scalar(out=kh[:, :], in0=hsc[:, :],
                                                scalar1=MAGIC, scalar2=MAGIC,
                                                op0=OP.add, op1=OP.subtract)
                        nc.sync.dma_start(
                            out=kT[:, q * (CQ // 128):(q + 1) * (CQ // 128),
                                   :],
                            in_=kh[:, :], transpose=True)
                    return kT, so

                n_pair = max(1, MT // 2)
                for g in range(n_pair):
                    pair = [m for m in (2 * g, 2 * g + 1) if m < MT]
                    kso = [kh_quant(m) for m in pair]
                    po = {}
                    for mi in range(len(pair)):
                        for c in range(ND):
                            po[(mi, c)] = psp.tile([128, 512], F32, tag="ps",
                                                   name=f"po{g}_{mi}_{c}")
                    for k in range(KH):
                        wo = m2p.tile([128, D], BF16, tag="wo", bufs=3,
                                      name=f"wo{g}_{k}")
                        nc.sync.dma_start(
                            out=wo[:, :],
                            in_=oq_d[k].rearrange("r p f -> p r f"))
                        for mi in range(len(pair)):
                            for c in range(ND):
                                nc.tensor.matmul(
                                    po[(mi, c)][:, :],
                                    lhsT=kso[mi][0][:, k, :],
                                    rhs=wo[:, c * 512:(c + 1) * 512],
                                    start=(k == 0), stop=(k == KH - 1))
                    for mi, m in enumerate(pair):
                        for c in range(ND):
                            ot = m2p.tile([128, 512], F32, tag="ot", bufs=4,
                                          name=f"ot{g}_{mi}_{c}")
                            nc.scalar.activation(out=ot[:, :],
                                                 in_=po[(mi, c)][:, :],
                                                 func=AF.Copy,
                                                 scale=kso[mi][1][:, :])
                            nc.sync.dma_start(
                                out=Ov[m][:, c * 512:(c + 1) * 512],
                                in_=ot[:, :])


_NC_CACHE = {}


def _get_nc(T, D, H):
    key = (T, D, H)
    if key not in _NC_CACHE:
        _NC_CACHE[key] = _build(T, D, H)
    return _NC_CACHE[key]


def kernel(x, gate_w, gate_b, val_w, val_b, out_w, out_b, _trace=False):
    x = np.ascontiguousarray(np.asarray(x), dtype=np.float32)
    gate_w = np.ascontiguousarray(np.asarray(gate_w), dtype=np.float32)
    val_w = np.ascontiguousarray(np.asarray(val_w), dtype=np.float32)
    out_w = np.ascontiguousarray(np.asarray(out_w), dtype=np.float32)
    gate_b = np.asarray(gate_b)
    val_b = np.asarray(val_b)
    out_b = np.asarray(out_b)
    assert not np.any(gate_b) and not np.any(val_b), (
        "device kernel folds silu(y+b) with b=0; nonzero gate/val bias "
        "not supported")

    orig_shape = x.shape
    xf = x.reshape(-1, x.shape[-1])
    n_tok, d = xf.shape
    h = gate_w.shape[0]
    t_core = n_tok // N_CORES

    nc = _get_nc(t_core, d, h)
    in_maps = [
        {
            "x": xf[i * t_core:(i + 1) * t_core],
            "gate_w": gate_w,
            "val_w": val_w,
            "out_w": out_w,
        }
        for i in range(N_CORES)
    ]
    res = run_bass_kernel_spmd(nc, in_maps, core_ids=list(range(N_CORES)),
                               trace=_trace)
    out = np.concatenate([res.results[i]["out"] for i in range(N_CORES)],
                         axis=0)
    out = out + out_b[None, :].astype(np.float32)
    kernel._last_results = res
    return out.reshape(orig_shape)
